# revision 13
# baseline (speedup 1.0000x reference)
"""TRN2 Bass/Tile kernel for nn_Loss_58317065945194.

Loss: per-sample EMD with r=2 over C=10 channels:
    d = p - q; S = cumsum(d, axis=1); out = mean(sqrt(mean(S**2, axis=1)))

The cumsum is linear: S = Wp.T @ p - Wp.T @ q with Wp a block-diagonal of 12
upper-triangular 10x10 ones-matrices (12 samples x 10 channels on 120
partitions, 512 samples per matmul column block). The subtract is fused into
PSUM accumulation (lhsT = -Wp for q). Inputs quantized to fp8e4 host-side:
halves the DMA packet count (the DMA fabric here is packet-latency bound)
and the PE consumes fp8 natively; quantization noise averages out over 2M
samples (measured rel err ~1e-3 vs 2e-2 tolerance).

Per 6144-sample tile (43 tiles/core), batches of 4 tiles:
  - Tensor: 4x MMp (lhsT=Wp, fp8) then 4x MMq (lhsT=-Wp) -> psum_S banks;
            batching same-weight matmuls back-to-back hides LDWEIGHTS and
            amortizes the pipeline drain.
  - Scalar/Vector (alternating): sq = Square(psum_S) -> SBUF fp16
  - Tensor: 4x reduce matmuls (lhsT = [120,12] channel-sum selector, fp16)
            into 12-row stripes at psum_U[32j:32j+12], tile_position=(0,32j)
            -> four different 32-col groups of the PE run them concurrently.
            One psum_U bank per 4-tile group; stripe gaps zeroed once.
  - Scalar: per group: sqrt(U/10) + accum_out -> per-group partial
Host sums partials over cores/groups and divides by B.

Sharding: pure data-parallel over B across 8 cores; each core's shard laid
out host-side as [120, 44032] fp8 (tile-column-major, p|q halves), zero-pad
tail samples (contribute 0). DMAs in 6-tile super-blocks (6KB/partition).
"""

import sys

import numpy as np

if "/opt/trn_rl_repo" not in sys.path:
    sys.path.insert(0, "/opt/trn_rl_repo")

N_CORES = 8
B, C = 2097152, 10
BS = B // N_CORES        # samples per core shard (262144)
SPB = 12                 # sample-blocks per column (12 * C = 120 rows)
KP = SPB * C             # active partitions (120)
NW = 512                 # samples per block-row per tile (psum bank width)
TPS = SPB * NW           # samples per tile (6144)
NT = -(-BS // TPS)       # tiles per core (43)
SPT = NT * TPS           # padded samples per core (264192)
GRP = 4                  # tiles per psum_U bank (stripes at 32-row offsets)
BAT = GRP                # tiles per matmul batch
NG = -(-NT // GRP)       # sqrt groups per core (11)
SUP = 6                  # tiles per DMA super-block
NSUP = -(-NT // SUP)     # super-blocks (8)

_cache = {}


def _build_weights():
    """w8 [128,2,128] fp8: Wp (block-diag upper-tri) and -Wp.
    w16 [128,16] fp16: cols 0:12 = channel-sum selector."""
    import ml_dtypes

    wp = np.zeros((128, 2, 128), dtype=np.float32)
    w12 = np.zeros((128, 16), dtype=np.float16)
    for s in range(SPB):
        for c in range(C):
            for i in range(c, C):
                wp[10 * s + c, 0, 10 * s + i] = 1.0
                wp[10 * s + c, 1, 10 * s + i] = -1.0
            w12[10 * s + c, s] = 1.0
    return wp.astype(ml_dtypes.float8_e4m3), w12


def _build_program():
    import concourse.tile as tile
    from concourse import bacc, mybir

    f32, f16, f8 = mybir.dt.float32, mybir.dt.float16, mybir.dt.float8e4
    Act = mybir.ActivationFunctionType
    Alu = mybir.AluOpType

    nc = bacc.Bacc(
        "TRN2", target_bir_lowering=False, debug=False, num_devices=N_CORES
    )
    x_d = nc.dram_tensor("x", [KP, NT * 2 * NW], f8, kind="ExternalInput").ap()
    w8_d = nc.dram_tensor("w8", [128, 2, 128], f8, kind="ExternalInput").ap()
    w16_d = nc.dram_tensor("w16", [128, 16], f16, kind="ExternalInput").ap()
    o_d = nc.dram_tensor("partial", [128, NG], f32, kind="ExternalOutput").ap()

    SW = SUP * 2 * NW  # columns per super-block DMA (6144)
    NB = -(-NT // BAT)  # batches (11)

    with tile.TileContext(nc) as tc:
        with (
            tc.tile_pool(name="io", bufs=4) as io,
            tc.tile_pool(name="wgt", bufs=1) as wgt,
            tc.tile_pool(name="sqp", bufs=8) as sqp,
            tc.tile_pool(name="scp", bufs=8) as scp,
            tc.tile_pool(name="junk", bufs=2) as junkp,
            tc.tile_pool(name="accp", bufs=1) as accp,
            tc.tile_pool(name="psS", bufs=6, space="PSUM") as psS,
            tc.tile_pool(name="psU", bufs=1, space="PSUM") as psU,
        ):
            w8t = wgt.tile([128, 2, 128], f8)
            nc.sync.dma_start(w8t[:], w8_d)
            w16t = wgt.tile([128, 16], f16)
            nc.sync.dma_start(w16t[:], w16_d)
            acc = accp.tile([128, NG], f32)
            psu = [
                psU.tile([128, NW], f32, tag="U0", name="psu0"),
                psU.tile([128, NW], f32, tag="U1", name="psu1"),
            ]
            # stripe gaps (rows 32j+12..32j+31) must read as exactly 0 forever
            nc.vector.memset(psu[0][:], 0.0)
            nc.vector.memset(psu[1][:], 0.0)

            wp = w8t[:KP, 0, :KP]    # [120, 120] cumsum weights
            wq = w8t[:KP, 1, :KP]    # -Wp
            w12 = w16t[:KP, :SPB]    # [120, 12] channel-sum selector

            xts = {}
            next_sup = 0
            sqs = {}
            prev = None  # tiles of the previous batch awaiting reduce+sqrt
            for b in range(NB + 1):
                # prefetch super-blocks ~2 batches ahead
                while next_sup < NSUP and next_sup * SUP < min(
                    NT, (b + 3) * BAT
                ):
                    s = next_sup
                    c0 = s * SW
                    c1 = min(NT * 2 * NW, c0 + SW)
                    xt = io.tile([KP, SW], f8, tag="x")
                    # two row-halves: keeps >=2 DMA descriptors outstanding so
                    # the per-engine 512B-burst latency overlaps across them
                    nc.sync.dma_start(xt[: KP // 2, : c1 - c0], x_d[: KP // 2, c0:c1])
                    nc.sync.dma_start(
                        xt[KP // 2 :, : c1 - c0], x_d[KP // 2 :, c0:c1]
                    )
                    xts[s] = xt
                    next_sup += 1
                tiles = range(b * BAT, min(NT, (b + 1) * BAT))
                if b < NB:
                    pss = {}
                    for t in tiles:
                        sup, tt = divmod(t, SUP)
                        xt = xts[sup]
                        ps = psS.tile([128, NW], f32, tag="S")
                        pss[t] = ps
                        nc.tensor.matmul(
                            ps[:KP],
                            wp,
                            xt[:, tt * 2 * NW : tt * 2 * NW + NW],
                            start=True,
                            stop=False,
                        )
                    for t in tiles:
                        sup, tt = divmod(t, SUP)
                        xt = xts[sup]
                        nc.tensor.matmul(
                            pss[t][:KP],
                            wq,
                            xt[:, tt * 2 * NW + NW : (tt + 1) * 2 * NW],
                            start=False,
                            stop=True,
                        )
                    for t in tiles:
                        # three-way psum evacuation: only ACT can square
                        # straight from PSUM; DVE can only copy it out (single
                        # psum input), with the square done by GpSimd or ACT
                        sq = sqp.tile([KP, NW], f16, tag="sq")
                        if t % 3 == 0:
                            nc.scalar.activation(sq[:], pss[t][:KP], Act.Square)
                        else:
                            sc = scp.tile([KP, NW], f16, tag="sc")
                            nc.vector.tensor_copy(out=sc[:], in_=pss[t][:KP])
                            if t % 14 in (1, 8):
                                nc.scalar.activation(sq[:], sc[:], Act.Square)
                            else:
                                nc.gpsimd.tensor_tensor(
                                    sq[:], sc[:], sc[:], Alu.mult
                                )
                        sqs[t] = sq
                # reduce quad one batch behind: 4 col-groups run concurrently
                if prev is not None:
                    g = prev[0] // GRP
                    bank = psu[g % 2]
                    for t in prev:
                        j = t % GRP
                        nc.tensor.matmul(
                            bank[32 * j : 32 * j + SPB],
                            w12,
                            sqs.pop(t)[:],
                            start=True,
                            stop=True,
                            tile_position=(0, 32 * j),
                        )
                    rows = 32 * ((len(prev) - 1) % GRP) + SPB
                    jk = junkp.tile([128, NW], f16, tag="jk")
                    nc.scalar.activation(
                        jk[:rows],
                        bank[:rows],
                        Act.Sqrt,
                        scale=1.0 / C,
                        accum_out=acc[:rows, g : g + 1],
                    )
                prev = list(tiles) if b < NB else None
            nc.sync.dma_start(o_d, acc[:])
    nc.compile()
    return nc


def _make_in_maps(p, q):
    """Per core: x = [120, NT*1024] fp8e4.

    Row 10*s + c, cols [1024t, 1024t+512)      -> p[base+t*6144+s*512+n, c]
    Row 10*s + c, cols [1024t+512, 1024t+1024) -> q[same sample, c]
    """
    import ml_dtypes

    f8 = ml_dtypes.float8_e4m3
    w8, w16 = _build_weights()

    def lay(a):
        a = np.asarray(a, dtype=np.float32).reshape(B, C).astype(f8)
        a = a.reshape(N_CORES, BS, C)
        pad = np.zeros((N_CORES, SPT, C), dtype=f8)
        pad[:, :BS] = a
        # [core, t, s, n, c] -> [core, t, s, c, n] = [core, NT, 120, 512]
        v = pad.reshape(N_CORES, NT, SPB, NW, C).transpose(0, 1, 2, 4, 3)
        return np.ascontiguousarray(v).reshape(N_CORES, NT, KP, NW)

    vp, vq = lay(p), lay(q)
    # [core, NT, 2, 120, 512] -> [core, 120, NT, 2, 512] -> [core, 120, NT*1024]
    x = np.stack([vp, vq], axis=2).transpose(0, 3, 1, 2, 4)
    x = np.ascontiguousarray(x).reshape(N_CORES, KP, NT * 2 * NW)
    return [{"x": x[i], "w8": w8, "w16": w16} for i in range(N_CORES)]


def kernel(p, q, r):
    assert int(r) == 2, f"kernel specialized for r=2, got {r}"
    if "nc" not in _cache:
        _cache["nc"] = _build_program()
    nc = _cache["nc"]

    in_maps = _make_in_maps(p, q)

    from concourse.bass_utils import run_bass_kernel_spmd

    res = run_bass_kernel_spmd(nc, in_maps, list(range(N_CORES)))
    total = 0.0
    for r_ in res.results:
        total += r_["partial"].astype(np.float64).sum()
    return np.float32(total / B)
